# revision 1
# baseline (speedup 1.0000x reference)
"""Trainium2 Bass kernel for the GaussianRenderer problem.

Contract: kernel(data, opacity) -> img
  data:    (32, 512, 8) float32
  opacity: (512, 1)     float32
  returns  (32, 3, 64, 64) float32

Sharding: data-parallel over batch B=32 across 8 NeuronCores (4 images
per core); no collectives.

Per-core algorithm (all compute on device):
  sigma[n, p] is a rank-6 bilinear form: sigma = F[n, :6] @ G[:6, p]
  where G rows are the pixel-coordinate monomials [1, x, y, x^2, y^2, xy]
  with x, y integer in [-32, 31] (exactly representable in fp16). F is
  derived per gaussian on-device (tanh/sigmoid/sin on ScalarE, arithmetic
  on VectorE), split into fp16 hi/mid/lo parts and stacked K=18 so a
  single fp16 TensorE matmul per [128 gaussians x 512 pixels] tile yields
  fp32-accurate (negated) sigma. alpha = Exp(sigma_neg) runs on ScalarE
  reading PSUM directly, writing fp16 to SBUF. Blending is a second
  TensorE matmul contracting the 128-gaussian partition dim with
  opacity-scaled colors split hi/lo (lo placed at partitions 32-34 so the
  final combine is a legal aligned VectorE add), accumulated over the 4
  gaussian tiles in PSUM, then DMA'd out.
"""

import numpy as np

import concourse.bacc as bacc
import concourse.mybir as mybir
import concourse.tile as tile
from concourse import bass_utils
from concourse._compat import get_trn_type
from concourse.alu_op_type import AluOpType

F32 = mybir.dt.float32
F16 = mybir.dt.float16
AF = mybir.ActivationFunctionType

N_CORES = 8
B = 32
B_CORE = B // N_CORES  # 4 images per core
N = 512                # gaussians
NG = B_CORE * N        # gaussians handled per core
NT = 16                # gaussian tiles of 128 per core (4 img * 4 ntiles)
HW = 4096              # pixels per image (64 x 64)
PI = float(np.pi)

# pixel segments per (img, ntile): sized so the sigma PSUM tile (3 banks)
# double-buffers alongside the blend accumulator within the 8 PSUM banks.
SEGS = [(0, 1536), (1536, 1536), (3072, 1024)]


def host_constants():
    """G2 [18, 4096] fp16 (3 stacked copies of the monomial rows, for the
    hi/mid/lo K-stacking) + fp16 identity for the PE transpose."""
    xs = np.arange(64, dtype=np.float64) - 32.0
    Xg, Yg = np.meshgrid(xs, xs)  # [h, w]; row-major pixels p = h*64 + w
    G = np.stack(
        [np.ones_like(Xg), Xg, Yg, Xg * Xg, Yg * Yg, Xg * Yg], 0
    ).reshape(6, HW)
    G2 = np.concatenate([G, G, G], 0).astype(np.float16)  # [18, 4096]
    ident = np.eye(128, dtype=np.float16)
    return G2, ident


def build_program(reps=1, loop=0, skip_prep=False, skip_blend=False):
    import contextlib

    nc = bacc.Bacc(get_trn_type() or "TRN2", target_bir_lowering=False, debug=False)
    # host pre-permutes into the on-chip layouts so these DMAs are contiguous
    d_data = nc.dram_tensor("data", (128, 128), F32, kind="ExternalInput")
    d_opac = nc.dram_tensor("opacity", (128, 4), F32, kind="ExternalInput")
    d_g2 = nc.dram_tensor("gconst", (18, HW), F16, kind="ExternalInput")
    d_id = nc.dram_tensor("ident", (128, 128), F16, kind="ExternalInput")
    d_img = nc.dram_tensor("img", (B_CORE, 3, 64, 64), F32, kind="ExternalOutput")
    if skip_prep:
        d_f2c = nc.dram_tensor("f2c", (18, NT * 128), F16, kind="ExternalInput")
        d_c2c = nc.dram_tensor("c2c", (128, NT * 35), F16, kind="ExternalInput")

    with tile.TileContext(nc) as tc:
      _loop_kw = dict(
          hint_engines=(
              mybir.EngineType.PE,
              mybir.EngineType.Activation,
              mybir.EngineType.DVE,
              mybir.EngineType.SP,
              mybir.EngineType.Pool,
          )
      )
      with tc.For_i(0, loop, 1, **_loop_kw) if loop else contextlib.nullcontext():
       for rep in range(reps):
        _r = f"r{rep}_" if reps > 1 else ""
        with (
            tc.tile_pool(name=_r + "const", bufs=1) as constp,
            tc.tile_pool(name=_r + "prep", bufs=1) as prep,
            tc.tile_pool(name=_r + "alpha", bufs=2) as alphap,
            tc.tile_pool(name=_r + "outp", bufs=4) as outp,
        ):
            # ---- constants + inputs to SBUF ----
            g2 = constp.tile([18, HW], F16, tag="g2")
            nc.sync.dma_start(g2[:], d_g2[:])
            idt = constp.tile([128, 128], F16, tag="idt")
            nc.sync.dma_start(idt[:], d_id[:])
            d8 = constp.tile([128, 128], F32, tag="d8")  # [p, t*8+k]
            nc.sync.dma_start(d8[:], d_data[:])
            opac = constp.tile([128, 4], F32, tag="opac")  # [p, ntile]
            nc.sync.dma_start(opac[:], d_opac[:])

            if skip_prep:
                f2 = constp.tile([18, NT * 128], F16, tag="f2")
                nc.sync.dma_start(f2[:], d_f2c[:])
                c2 = constp.tile([128, NT * 35], F16, tag="c2")
                nc.sync.dma_start(c2[:], d_c2c[:])
            else:
                d8v = d8.rearrange("p (t k) -> p t k", k=8)

                def field(k):  # [128, 16] strided view of input field k
                    return d8v[:, :, k]

                def t16(tag):
                    return prep.tile([128, 16], F32, tag=tag, name=_r + tag)

                # ---- per-gaussian preprocessing ([128, 16] fp32 tiles) ----
                # theta = 2*pi*sigmoid(d4). Build sin/cos(2*theta) from
                # half-angle pieces so ACT Sin only sees args in (-pi/2, pi):
                #   s1 = sin(pi*sg), nc1 = sin(pi*sg - pi/2) = -cos(pi*sg)
                #   sin(theta) = -2*s1*nc1, cos(theta) = 1 - 2*s1^2, then
                #   double-angle once more for sin/cos(2*theta).
                sg = t16("sg")
                nc.scalar.activation(sg[:], field(4), AF.Sigmoid)
                a1 = t16("a1")
                nc.vector.tensor_scalar_mul(a1[:], sg[:], PI)
                s1 = t16("s1")
                nc.scalar.activation(s1[:], a1[:], AF.Sin)
                a2 = t16("a2")
                nc.vector.tensor_scalar(
                    a2[:], sg[:], PI, -PI / 2, AluOpType.mult, AluOpType.add
                )
                nc1 = t16("nc1")
                nc.scalar.activation(nc1[:], a2[:], AF.Sin)
                sth = t16("sth")  # sin(theta)
                nc.vector.scalar_tensor_tensor(
                    sth[:], s1[:], -2.0, nc1[:], AluOpType.mult, AluOpType.mult
                )
                cth = t16("cth")  # cos(theta) = 1 - 2*s1^2
                nc.vector.tensor_tensor(cth[:], s1[:], s1[:], AluOpType.mult)
                nc.vector.tensor_scalar(
                    cth[:], cth[:], -2.0, 1.0, AluOpType.mult, AluOpType.add
                )
                s2t = t16("s2t")  # sin(2*theta) = 2*sin(theta)*cos(theta)
                nc.vector.scalar_tensor_tensor(
                    s2t[:], sth[:], 2.0, cth[:], AluOpType.mult, AluOpType.mult
                )
                c2t = t16("c2t")  # cos(2*theta) = 1 - 2*sin(theta)^2
                nc.vector.tensor_tensor(c2t[:], sth[:], sth[:], AluOpType.mult)
                nc.vector.tensor_scalar(
                    c2t[:], c2t[:], -2.0, 1.0, AluOpType.mult, AluOpType.add
                )

                # centers (global shift -32): ex = 32*tanh(d0) - 0.5
                ex = t16("ex")
                nc.scalar.activation(ex[:], field(0), AF.Tanh)
                nc.vector.tensor_scalar(
                    ex[:], ex[:], 32.0, -0.5, AluOpType.mult, AluOpType.add
                )
                ey = t16("ey")
                nc.scalar.activation(ey[:], field(1), AF.Tanh)
                nc.vector.tensor_scalar(
                    ey[:], ey[:], 32.0, -0.5, AluOpType.mult, AluOpType.add
                )

                # scales: h0 = 0.5*(|d2|+0.3)^2, h1 = 0.5*(|d3|+0.3)^2
                s0 = t16("s0")
                nc.scalar.activation(s0[:], field(2), AF.Abs)
                nc.vector.tensor_scalar_add(s0[:], s0[:], 0.3)
                s1s = t16("s1s")
                nc.scalar.activation(s1s[:], field(3), AF.Abs)
                nc.vector.tensor_scalar_add(s1s[:], s1s[:], 0.3)
                h0 = t16("h0")
                nc.vector.tensor_tensor(h0[:], s0[:], s0[:], AluOpType.mult)
                nc.vector.tensor_scalar_mul(h0[:], h0[:], 0.5)
                h1 = t16("h1")
                nc.vector.tensor_tensor(h1[:], s1s[:], s1s[:], AluOpType.mult)
                nc.vector.tensor_scalar_mul(h1[:], h1[:], 0.5)

                sum5 = t16("sum5")  # 0.5*(s0^2+s1^2)
                nc.vector.tensor_tensor(sum5[:], h0[:], h1[:], AluOpType.add)
                dif5 = t16("dif5")  # 0.5*(s0^2-s1^2)
                nc.vector.tensor_tensor(dif5[:], h0[:], h1[:], AluOpType.subtract)

                # covariance entries
                dc = t16("dc")
                nc.vector.tensor_tensor(dc[:], dif5[:], c2t[:], AluOpType.mult)
                cov_a = t16("cov_a")
                nc.vector.tensor_tensor(cov_a[:], sum5[:], dc[:], AluOpType.add)
                cov_c = t16("cov_c")
                nc.vector.tensor_tensor(cov_c[:], sum5[:], dc[:], AluOpType.subtract)
                cov_b = t16("cov_b")
                nc.vector.tensor_tensor(cov_b[:], dif5[:], s2t[:], AluOpType.mult)

                det = t16("det")
                nc.vector.tensor_tensor(det[:], cov_a[:], cov_c[:], AluOpType.mult)
                bb = t16("bb")
                nc.vector.tensor_tensor(bb[:], cov_b[:], cov_b[:], AluOpType.mult)
                nc.vector.tensor_tensor(det[:], det[:], bb[:], AluOpType.subtract)

                # conic: ca = cov_c/det, cc = cov_a/det, cbn = cov_b/det (= -con_b)
                inv = t16("inv")
                nc.vector.reciprocal(inv[:], det[:])
                ca = t16("ca")
                nc.vector.tensor_tensor(ca[:], cov_c[:], inv[:], AluOpType.mult)
                cc = t16("cc")
                nc.vector.tensor_tensor(cc[:], cov_a[:], inv[:], AluOpType.mult)
                cbn = t16("cbn")
                nc.vector.tensor_tensor(cbn[:], cov_b[:], inv[:], AluOpType.mult)

                # ---- F rows (negated for exp), written into Fc [128, 96] ----
                Fc = prep.tile([128, 96], F32, tag="Fc")
                Fv = Fc.rearrange("p (t k) -> p t k", k=6)

                exq = t16("exq")
                nc.vector.tensor_tensor(exq[:], ex[:], ex[:], AluOpType.mult)
                eyq = t16("eyq")
                nc.vector.tensor_tensor(eyq[:], ey[:], ey[:], AluOpType.mult)
                exey = t16("exey")
                nc.vector.tensor_tensor(exey[:], ex[:], ey[:], AluOpType.mult)

                # f0 = -0.5*ca*exq - 0.5*cc*eyq + cbn*exey
                t_a = t16("t_a")
                nc.vector.tensor_tensor(t_a[:], ca[:], exq[:], AluOpType.mult)
                t_b = t16("t_b")
                nc.vector.tensor_tensor(t_b[:], cc[:], eyq[:], AluOpType.mult)
                nc.vector.tensor_tensor(t_a[:], t_a[:], t_b[:], AluOpType.add)
                nc.vector.tensor_scalar_mul(t_a[:], t_a[:], -0.5)
                t_c = t16("t_c")
                nc.vector.tensor_tensor(t_c[:], cbn[:], exey[:], AluOpType.mult)
                nc.vector.tensor_tensor(Fv[:, :, 0], t_a[:], t_c[:], AluOpType.add)

                # f_x = ca*ex - cbn*ey ; f_y = cc*ey - cbn*ex
                nc.vector.tensor_tensor(t_a[:], ca[:], ex[:], AluOpType.mult)
                nc.vector.tensor_tensor(t_b[:], cbn[:], ey[:], AluOpType.mult)
                nc.vector.tensor_tensor(Fv[:, :, 1], t_a[:], t_b[:], AluOpType.subtract)
                nc.vector.tensor_tensor(t_a[:], cc[:], ey[:], AluOpType.mult)
                nc.vector.tensor_tensor(t_b[:], cbn[:], ex[:], AluOpType.mult)
                nc.vector.tensor_tensor(Fv[:, :, 2], t_a[:], t_b[:], AluOpType.subtract)

                # f_x2 = -0.5*ca ; f_y2 = -0.5*cc ; f_xy = +cbn
                nc.vector.tensor_scalar_mul(Fv[:, :, 3], ca[:], -0.5)
                nc.vector.tensor_scalar_mul(Fv[:, :, 4], cc[:], -0.5)
                nc.vector.tensor_scalar_mul(Fv[:, :, 5], cbn[:], 1.0)

                # ---- split F into fp16 hi/mid/lo, interleaved [128, 16*18] ----
                fall = prep.tile([128, NT * 18], F16, tag="fall")
                fv = fall.rearrange("p (t s) -> p t s", s=18)
                Fc6 = Fc.rearrange("p (t k) -> p t k", k=6)
                nc.vector.tensor_copy(fv[:, :, 0:6], Fc6[:, :, :])
                r1 = prep.tile([128, 96], F32, tag="r1")
                r16 = r1.rearrange("p (t k) -> p t k", k=6)
                nc.vector.tensor_tensor(
                    r16[:, :, :], Fc6[:, :, :], fv[:, :, 0:6], AluOpType.subtract
                )
                nc.vector.tensor_copy(fv[:, :, 6:12], r16[:, :, :])
                r2 = prep.tile([128, 96], F32, tag="r2")
                r26 = r2.rearrange("p (t k) -> p t k", k=6)
                nc.vector.tensor_tensor(
                    r26[:, :, :], r16[:, :, :], fv[:, :, 6:12], AluOpType.subtract
                )
                nc.vector.tensor_copy(fv[:, :, 12:18], r26[:, :, :])

                # ---- per-tile transpose: [128, 18] -> psum [18, 128] -> f2 ----
                f2 = constp.tile([18, NT * 128], F16, tag="f2")  # matmul weights
                with tc.tile_pool(name=_r + "prepps", bufs=2, space="PSUM") as prepps:
                    for t in range(NT):
                        tp = prepps.tile([18, 128], F16, tag="tp", name=f"{_r}tp{t}")
                        nc.tensor.transpose(tp[:], fall[:, t * 18 : (t + 1) * 18], idt[:])
                        nc.vector.tensor_copy(f2[:, t * 128 : (t + 1) * 128], tp[:])

                # ---- colors * opacity, split hi/lo -> c2 [128, 16*35] fp16 ----
                # hi at cols t*35+{0,1,2}, lo at t*35+{32,33,34}: the blend
                # matmul then lands lo rows at PSUM partitions 32-34, which the
                # 32-aligned VectorE combine can read.
                opac_b = opac[:].unsqueeze(1).broadcast_to([128, 4, 4])
                cP = prep.tile([128, 48], F32, tag="cP")
                cPv = cP.rearrange("p (t k) -> p t k", k=3)
                cP4 = cP.rearrange("p (i n k) -> p i n k", n=4, k=3)
                d84 = d8.rearrange("p (i n k) -> p i n k", n=4, k=8)
                for k in range(3):
                    nc.vector.tensor_tensor(
                        cP4[:, :, :, k], d84[:, :, :, 5 + k], opac_b, AluOpType.mult
                    )
                c2 = constp.tile([128, NT * 35], F16, tag="c2")
                nc.vector.memset(c2[:], 0.0)
                c2v = c2.rearrange("p (t s) -> p t s", s=35)
                nc.vector.tensor_copy(c2v[:, :, 0:3], cPv[:, :, :])
                chi32 = prep.tile([128, 48], F32, tag="chi32")
                nc.vector.tensor_copy(
                    chi32.rearrange("p (t k) -> p t k", k=3)[:, :, :], c2v[:, :, 0:3]
                )
                rlo = prep.tile([128, 48], F32, tag="rlo")
                nc.vector.tensor_tensor(rlo[:], cP[:], chi32[:], AluOpType.subtract)
                nc.vector.tensor_copy(
                    c2v[:, :, 32:35], rlo.rearrange("p (t k) -> p t k", k=3)[:, :, :]
                )


            # ---- main loop ----
            with (
                tc.tile_pool(name=_r + "sigps", bufs=2, space="PSUM") as sigps,
                tc.tile_pool(name=_r + "blps", bufs=2, space="PSUM") as blps,
            ):
                for img in range(B_CORE):
                    al = alphap.tile([128, 4 * HW], F16, tag="al", name=f"{_r}al{img}")
                    for nt in range(4):
                        t = img * 4 + nt
                        w = f2[:, t * 128 : (t + 1) * 128]
                        for off, ln in SEGS:
                            sps = sigps.tile(
                                [128, ln], F32, tag="sig", name=f"{_r}sig{img}_{nt}_{off}"
                            )
                            for q in range(0, ln, 512):
                                nc.tensor.matmul(
                                    sps[:, q : q + 512],
                                    w,
                                    g2[:, off + q : off + q + 512],
                                    start=True,
                                    stop=True,
                                )
                            nc.scalar.activation(
                                al[:, nt * HW + off : nt * HW + off + ln],
                                sps[:],
                                AF.Exp,
                            )
                    for ch in range(8):
                        if skip_blend:
                            continue
                        bps = blps.tile(
                            [35, 512], F32, tag="bl", name=f"{_r}bl{img}_{ch}"
                        )
                        for nt in range(4):
                            t = img * 4 + nt
                            nc.tensor.matmul(
                                bps[:],
                                c2[:, t * 35 : t * 35 + 35],
                                al[:, nt * HW + ch * 512 : nt * HW + ch * 512 + 512],
                                start=(nt == 0),
                                stop=(nt == 3),
                            )
                        ot = outp.tile([3, 512], F32, tag="ot", name=f"{_r}ot{img}_{ch}")
                        nc.vector.tensor_copy(ot[:], bps[32:35, :])
                        nc.vector.tensor_tensor(
                            ot[:], bps[0:3, :], ot[:], AluOpType.add
                        )
                        nc.sync.dma_start(
                            d_img[img, :, ch * 8 : (ch + 1) * 8, :].rearrange(
                                "c h w -> c (h w)"
                            ),
                            ot[:],
                        )

    nc.compile()
    return nc


_NC_CACHE = None


def _get_program():
    global _NC_CACHE
    if _NC_CACHE is None:
        _NC_CACHE = build_program()
    return _NC_CACHE


def make_in_maps(data, opacity):
    data = np.ascontiguousarray(np.asarray(data, dtype=np.float32))
    opacity = np.ascontiguousarray(np.asarray(opacity, dtype=np.float32))
    G2, ident = host_constants()
    in_maps = []
    op_pt = np.ascontiguousarray(opacity.reshape(4, 128).T)  # [p, ntile]
    for c in range(N_CORES):
        dc = data[c * B_CORE : (c + 1) * B_CORE].reshape(NG, 8)
        # device layout d8[p, t*8+k] = data[t*128+p, k]
        d8 = np.ascontiguousarray(
            dc.reshape(NT, 128, 8).transpose(1, 0, 2).reshape(128, 128)
        )
        in_maps.append(
            {"data": d8, "opacity": op_pt, "gconst": G2, "ident": ident}
        )
    return in_maps


def kernel(data, opacity):
    nc = _get_program()
    in_maps = make_in_maps(data, opacity)
    res = bass_utils.run_bass_kernel_spmd(nc, in_maps, core_ids=list(range(N_CORES)))
    out = np.concatenate(
        [res.results[c]["img"] for c in range(N_CORES)], axis=0
    ).astype(np.float32)
    return out



# revision 34
# speedup vs baseline: 39.1122x; 39.1122x over previous
"""Trainium2 Bass kernel for the GaussianRenderer problem.

Contract: kernel(data, opacity) -> img
  data:    (32, 512, 8) float32
  opacity: (512, 1)     float32
  returns  (32, 3, 64, 64) float32

Sharding: data-parallel over batch B=32 across 8 NeuronCores (4 images
per core); no collectives.

Algorithm (sparse region rendering):
  The image is split into 8 disjoint regions of 8 rows (512 px). On the
  host, each gaussian is assigned to every region its footprint touches
  (|dy| <= sqrt(2 ln(1/eps) cov_yy), eps=1e-4); contributions outside are
  below eps and dropped. Per region, the (image, gaussian) slot lists of
  the core's 4 images are concatenated and padded to a multiple of 128
  (the partition dim). Per 128-slot tile:
    sigma[slot, px] is a rank-6 bilinear form sigma = F[slot,:6] @ G[:6,px]
  with G rows the pixel monomials [1,x,y,x^2,y^2,xy] (x,y in [-32,31],
  exact in fp16). F is derived per slot on-device (tanh/sigmoid/sin on
  ScalarE, arithmetic on VectorE), split fp16 hi/mid/lo and stacked K=18
  so one fp16 TensorE matmul [18,128]x[18,512] yields fp32-accurate
  negated sigma in PSUM. alpha = Exp(sigma_neg) on ScalarE (PSUM->SBUF,
  fp16, two tiles per activation). Blending contracts the 128-slot
  partition dim with block-diagonal weights c2[slot, 3*img(slot)+c] =
  color*opacity (opacity+image routing arrives as a host mask), so one
  PSUM tile [12, 512] accumulates all 4 images' channels for the region
  across its tiles. Regions are disjoint -> no cross-region adds; a DVE
  copy moves each finished region to SBUF and one DMA writes the core's
  (4,3,64,64) output.
"""

import numpy as np

import concourse.bacc as bacc
import concourse.mybir as mybir
import concourse.tile as tile
from concourse import bass_utils
from concourse._compat import get_trn_type
from concourse.alu_op_type import AluOpType

F32 = mybir.dt.float32
F16 = mybir.dt.float16
AF = mybir.ActivationFunctionType

N_CORES = 8
B = 32
B_CORE = B // N_CORES  # 4 images per core
N = 512                # gaussians per image
HW = 4096              # pixels per image (64 x 64)
NREG = 8               # 8-row regions per image
RPX = 512              # pixels per region
PI = float(np.pi)
EPS = 1e-4             # alpha cutoff for footprint assignment
KCUT = float(np.sqrt(2.0 * np.log(1.0 / EPS)))


def host_constants():
    """G2 [12, 4096] fp16 (2 stacked copies of the monomial rows, for the
    hi/lo K-stacking) + fp16 identity for the PE transpose."""
    xs = np.arange(64, dtype=np.float64) - 32.0
    Xg, Yg = np.meshgrid(xs, xs)  # [h, w]; row-major pixels p = h*64 + w
    G = np.stack(
        [np.ones_like(Xg), Xg, Yg, Xg * Xg, Yg * Yg, Xg * Yg], 0
    ).reshape(6, HW)
    G2 = np.concatenate([G, G], 0).astype(np.float16)  # [12, 4096]
    ident = np.eye(128, dtype=np.float16)
    return G2, ident


def footprints(data):
    """Per (image, gaussian): centers (px, py) and half-extents (rx, ry)
    such that alpha < EPS strictly outside the box."""
    d = np.asarray(data, np.float64)
    px = 0.5 * ((np.tanh(d[..., 0]) + 1.0) * 64 - 1.0)
    py = 0.5 * ((np.tanh(d[..., 1]) + 1.0) * 64 - 1.0)
    s0 = np.abs(d[..., 2]) + 0.3
    s1 = np.abs(d[..., 3]) + 0.3
    th = 1.0 / (1.0 + np.exp(-d[..., 4])) * (2.0 * PI)
    c, s = np.cos(th), np.sin(th)
    cov_xx = c * c * s0 * s0 + s * s * s1 * s1
    cov_yy = s * s * s0 * s0 + c * c * s1 * s1
    rx = KCUT * np.sqrt(cov_xx)
    ry = KCUT * np.sqrt(cov_yy)
    return px, py, rx, ry


RX_WIDE = 10.0  # column-wide gaussians go first, into the full-width tile


def region_slots(data, core, r, fp=None):
    """Ordered slot list [(img_local, gauss)] of region r for a core:
    every gaussian whose row footprint intersects rows [8r, 8r+8).
    Column-wide gaussians sort first (absorbed by the full-width first
    tile); the rest sort by center column for tight column windows."""
    px, py, rx, ry = fp if fp is not None else footprints(data)
    slots = []
    for i in range(B_CORE):
        b = core * B_CORE + i
        m = (py[b] + ry[b] >= 8 * r) & (py[b] - ry[b] < 8 * r + 8)
        slots.extend(
            (rx[b, g] < RX_WIDE, float(px[b, g]), i, int(g))
            for g in np.nonzero(m)[0]
        )
    slots.sort()
    return [(i, g) for _, _, i, g in slots]


def layout(data):
    """Uniform (across cores) tiles-per-region + per-tile column windows
    from the actual input. Returns (tiles_r, cwin) with cwin[t]=(c0, w)."""
    fp = footprints(data)
    px, py, rx, ry = fp
    all_slots = [
        [region_slots(data, c, r, fp) for r in range(NREG)] for c in range(N_CORES)
    ]
    tiles_r = tuple(
        int(np.ceil(max(len(all_slots[c][r]) for c in range(N_CORES)) / 128))
        for r in range(NREG)
    )
    cwin = []
    for r in range(NREG):
        for k in range(tiles_r[r]):
            if k == 0:
                # the start=True tile must cover the full accumulator so
                # every PSUM element is written (has_written semantics)
                cwin.append((0, 64))
                continue
            c0, c1 = 64, 0
            for c in range(N_CORES):
                for i, g in all_slots[c][r][k * 128 : (k + 1) * 128]:
                    b = c * B_CORE + i
                    c0 = min(c0, px[b, g] - rx[b, g])
                    c1 = max(c1, px[b, g] + rx[b, g])
            c0 = int(np.clip(np.floor(c0), 0, 64)) & ~1
            c1 = min((int(np.clip(np.ceil(c1), 0, 64)) + 1) & ~1, 64)
            c1 = max(c1, c0 + 16)  # floor width (AP/cacheline friendliness)
            if c1 > 64:
                c0, c1 = 48, 64
            cwin.append((c0, c1 - c0))
    return tiles_r, tuple(cwin)


def build_program(tiles_r, cwin=None, reps=1, loop=0, prepb=1, sigb=4, blb=3, look=3):
    import contextlib

    tiles_r = tuple(tiles_r)
    T = sum(tiles_r)  # total 128-slot tiles per core
    if cwin is None:
        cwin = ((0, 64),) * T
    nc = bacc.Bacc(get_trn_type() or "TRN2", target_bir_lowering=False, debug=False)
    # const APs for ACT biases (only 0.0/1.0 are pre-registered)
    for _cv, _cn in ((0.3, "0p3"), (-PI / 2, "mhpi"), (-0.5, "mhalf")):
        _ct = nc.alloc_sbuf_tensor(f"const-f32-{_cn}", [128, 1], F32)
        nc.gpsimd.memset(_ct.ap(), _cv)
        nc.const_aps.aps[(F32, _cv)] = _ct.ap()
    d_data = nc.dram_tensor("data", (128, T * 8), F32, kind="ExternalInput")
    d_mask = nc.dram_tensor("mask", (128, T * 12), F16, kind="ExternalInput")
    d_g2 = nc.dram_tensor("gconst", (12, HW), F16, kind="ExternalInput")
    d_id = nc.dram_tensor("ident", (128, 128), F16, kind="ExternalInput")
    d_img = nc.dram_tensor("img", (B_CORE, 3, 64, 64), F32, kind="ExternalOutput")

    with tile.TileContext(nc) as tc:
      _loop_kw = dict(
          hint_engines=(
              mybir.EngineType.PE,
              mybir.EngineType.Activation,
              mybir.EngineType.DVE,
              mybir.EngineType.SP,
              mybir.EngineType.Pool,
          )
      )
      with tc.For_i(0, loop, 1, **_loop_kw) if loop else contextlib.nullcontext():
       for rep in range(reps):
        _r = f"r{rep}_" if reps > 1 else ""
        with (
            tc.tile_pool(name=_r + "const", bufs=1) as constp,
            tc.tile_pool(name=_r + "prep", bufs=1) as prep,
            tc.tile_pool(name=_r + "alpha", bufs=3) as alphap,
            tc.tile_pool(name=_r + "outp", bufs=1) as outp,
        ):
            # ---- constants + inputs to SBUF (d8 first: it gates the
            # whole prep chain; the rest is needed only later) ----
            d8 = constp.tile([128, T * 8], F32, tag="d8")  # [p, t*8+k]
            nc.sync.dma_start(d8[:], d_data[:])
            idt = constp.tile([128, 128], F16, tag="idt")
            nc.sync.dma_start(idt[:], d_id[:])
            g2 = constp.tile([12, HW], F16, tag="g2")
            nc.sync.dma_start(g2[:], d_g2[:])
            msk = constp.tile([128, T * 12], F16, tag="msk")
            nc.sync.dma_start(msk[:], d_mask[:])

            d8v = d8.rearrange("p (t k) -> p t k", k=8)

            def field(k):  # [128, T] strided view of input field k
                return d8v[:, :, k]

            def tT(tag):
                return prep.tile([128, T], F32, tag=tag, name=_r + tag)

            # ---- per-slot preprocessing ([128, T] fp32 tiles) ----
            # theta = 2*pi*sigmoid(d4). Build sin/cos(2*theta) from
            # half-angle pieces so ACT Sin only sees args in (-pi/2, pi):
            #   s1 = sin(pi*sg), nc1 = sin(pi*sg - pi/2) = -cos(pi*sg)
            #   sin(theta) = -2*s1*nc1, cos(theta) = 1 - 2*s1^2, then
            #   double-angle once more for sin/cos(2*theta).
            sg = tT("sg")
            nc.scalar.activation(sg[:], field(4), AF.Sigmoid)
            s1 = tT("s1")
            nc.scalar.activation(s1[:], sg[:], AF.Sin, scale=PI)
            nc1 = tT("nc1")
            nc.scalar.activation(nc1[:], sg[:], AF.Sin, scale=PI, bias=-PI / 2)
            sth = tT("sth")  # sin(theta)
            nc.vector.scalar_tensor_tensor(
                sth[:], s1[:], -2.0, nc1[:], AluOpType.mult, AluOpType.mult
            )
            cth = tT("cth")  # cos(theta) = 1 - 2*s1^2
            nc.vector.tensor_tensor(cth[:], s1[:], s1[:], AluOpType.mult)
            nc.vector.tensor_scalar(
                cth[:], cth[:], -2.0, 1.0, AluOpType.mult, AluOpType.add
            )
            s2t = tT("s2t")  # sin(2*theta) = 2*sin(theta)*cos(theta)
            nc.vector.scalar_tensor_tensor(
                s2t[:], sth[:], 2.0, cth[:], AluOpType.mult, AluOpType.mult
            )
            c2t = tT("c2t")  # cos(2*theta) = 1 - 2*sin(theta)^2
            nc.vector.tensor_tensor(c2t[:], sth[:], sth[:], AluOpType.mult)
            nc.vector.tensor_scalar(
                c2t[:], c2t[:], -2.0, 1.0, AluOpType.mult, AluOpType.add
            )

            # centers (global shift -32): ex = 32*tanh(d0) - 0.5
            th0 = tT("th0")
            nc.scalar.activation(th0[:], field(0), AF.Tanh)
            th1 = tT("th1")
            nc.scalar.activation(th1[:], field(1), AF.Tanh)
            ex = tT("ex")
            nc.vector.tensor_scalar(
                ex[:], th0[:], 32.0, -0.5, AluOpType.mult, AluOpType.add
            )
            ey = tT("ey")
            nc.vector.tensor_scalar(
                ey[:], th1[:], 32.0, -0.5, AluOpType.mult, AluOpType.add
            )

            # scales (on ACT: Abs and Square are filler functions present in
            # every table set): q0 = (|d2|+0.3)^2, q1 = (|d3|+0.3)^2
            s0 = tT("s0")
            nc.scalar.activation(s0[:], field(2), AF.Abs)
            s1s = tT("s1s")
            nc.scalar.activation(s1s[:], field(3), AF.Abs)

            # conic directly: R diag(1/s0^2, 1/s1^2) R^T (det of the
            # covariance is rotation-free, so no det/cofactor chain).
            # is0 = 1/s0^2, is1 = 1/s1^2; si = is0+is1, di = is0-is1;
            # 2*con_a = si + di*c2t; 2*con_c = si - di*c2t; cb2 = di*s2t
            # (= -2*cbn where cbn = -con_b).
            q0 = tT("q0")
            nc.scalar.activation(q0[:], s0[:], AF.Square, bias=0.3)
            q1 = tT("q1")
            nc.scalar.activation(q1[:], s1s[:], AF.Square, bias=0.3)
            is0 = tT("is0")
            nc.vector.reciprocal(is0[:], q0[:])
            is1 = tT("is1")
            nc.vector.reciprocal(is1[:], q1[:])
            si = tT("si")
            nc.vector.tensor_tensor(si[:], is0[:], is1[:], AluOpType.add)
            di = tT("di")
            nc.vector.tensor_tensor(di[:], is0[:], is1[:], AluOpType.subtract)
            dc2 = tT("dc2")
            nc.vector.tensor_tensor(dc2[:], di[:], c2t[:], AluOpType.mult)
            ca = tT("ca")  # 2*con_a
            nc.vector.tensor_tensor(ca[:], si[:], dc2[:], AluOpType.add)
            cc = tT("cc")  # 2*con_c
            nc.vector.tensor_tensor(cc[:], si[:], dc2[:], AluOpType.subtract)
            cb2 = tT("cb2")  # -2*cbn = 2*con_b
            nc.vector.tensor_tensor(cb2[:], di[:], s2t[:], AluOpType.mult)

            # ---- F rows (negated for exp), written into Fc [128, T*6] ----
            Fc = prep.tile([128, T * 6], F32, tag="Fc")
            Fv = Fc.rearrange("p (t k) -> p t k", k=6)

            exq = tT("exq")
            nc.scalar.activation(exq[:], th0[:], AF.Square, scale=32.0, bias=-0.5)
            eyq = tT("eyq")
            nc.scalar.activation(eyq[:], th1[:], AF.Square, scale=32.0, bias=-0.5)
            exey = tT("exey")
            nc.vector.tensor_tensor(exey[:], ex[:], ey[:], AluOpType.mult)

            # F' rows are 2x the negated-sigma coefficients (the doubled
            # conic is used directly); the Exp activation applies the 0.5
            # via its free affine scale.
            # f0' = -0.5*(ca*exq + cc*eyq) - cb2*exey
            t_a = tT("t_a")
            nc.vector.tensor_tensor(t_a[:], ca[:], exq[:], AluOpType.mult)
            t_b = tT("t_b")
            nc.vector.tensor_tensor(t_b[:], cc[:], eyq[:], AluOpType.mult)
            nc.vector.tensor_tensor(t_a[:], t_a[:], t_b[:], AluOpType.add)
            nc.vector.tensor_scalar_mul(t_a[:], t_a[:], -0.5)
            t_c = tT("t_c")
            nc.vector.tensor_tensor(t_c[:], cb2[:], exey[:], AluOpType.mult)
            nc.vector.tensor_tensor(Fv[:, :, 0], t_a[:], t_c[:], AluOpType.subtract)

            # f_x' = ca*ex + cb2*ey ; f_y' = cc*ey + cb2*ex
            nc.vector.tensor_tensor(t_a[:], ca[:], ex[:], AluOpType.mult)
            nc.vector.tensor_tensor(t_b[:], cb2[:], ey[:], AluOpType.mult)
            nc.vector.tensor_tensor(Fv[:, :, 1], t_a[:], t_b[:], AluOpType.add)
            nc.vector.tensor_tensor(t_a[:], cc[:], ey[:], AluOpType.mult)
            nc.vector.tensor_tensor(t_b[:], cb2[:], ex[:], AluOpType.mult)
            nc.vector.tensor_tensor(Fv[:, :, 2], t_a[:], t_b[:], AluOpType.add)

            # f_x2' = -0.5*ca ; f_y2' = -0.5*cc ; f_xy' = -cb2
            nc.vector.tensor_scalar_mul(Fv[:, :, 3], ca[:], -0.5)
            nc.vector.tensor_scalar_mul(Fv[:, :, 4], cc[:], -0.5)
            nc.vector.tensor_scalar_mul(Fv[:, :, 5], cb2[:], -1.0)

            # ---- split F into fp16 hi/lo, interleaved [128, T*12] ----
            fall = prep.tile([128, T * 12], F16, tag="fall")
            fv = fall.rearrange("p (t s) -> p t s", s=12)
            Fc6 = Fc.rearrange("p (t k) -> p t k", k=6)
            nc.vector.tensor_copy(fv[:, :, 0:6], Fc6[:, :, :])
            nc.vector.tensor_tensor(
                fv[:, :, 6:12], Fc6[:, :, :], fv[:, :, 0:6], AluOpType.subtract
            )

            # ---- colors * mask -> c2 [128, T*12] fp16 (block-diag weights) ----
            c2 = constp.tile([128, T * 12], F16, tag="c2")
            c2v = c2.rearrange("p (t i k) -> p t i k", i=4, k=3)
            mv = msk.rearrange("p (t i k) -> p t i k", i=4, k=3)
            cb = d8v[:, :, 5:8].unsqueeze(2).broadcast_to([128, T, 4, 3])
            nc.vector.tensor_tensor(c2v[:, :, :, :], cb, mv[:, :, :, :], AluOpType.mult)

            # ---- main loop: regions of 8 rows ----
            # Steps = pairs over the FLAT tile list (pairs may span region
            # boundaries) so every exp is a uniform [128, 1024] op — a
            # shorter odd exp breaks the pipeline rhythm and exposes the
            # sem/dispatch latency. Blend matmuls run one step behind the
            # sigma/exp stream so PE always has independent sigma work
            # queued while blends wait on the latest exp. PE transposes
            # ([128,12] -> psum [12, 8*128] -> f2) are interleaved
            # just-in-time ahead of the sigma stream.
            treg = []  # tile index -> (region, is_first, is_last)
            base = 0
            for r in range(NREG):
                nt = tiles_r[r]
                for k in range(nt):
                    treg.append((r, k == 0, k == nt - 1))
                base += nt
            # greedy-pack tiles into steps of <= 512 psum columns (one
            # bank) so the sigma pool can run 4 bufs deep within the
            # 8-bank budget (4 sig + 1 prep + 2 blend)
            steps = []
            cur, cw_sum = [], 0
            for t in range(T):
                w8 = 8 * cwin[t][1]
                if cur and cw_sum + w8 > RPX:
                    steps.append(cur)
                    cur, cw_sum = [], 0
                cur.append(t)
                cw_sum += w8
            if cur:
                steps.append(cur)

            TB = 8  # tiles per transpose batch (psum: 8*128 fp16 = 1 bank)
            f2 = constp.tile([12, T * 128], F16, tag="f2")  # matmul weights
            stag = outp.tile([12, HW], F32, tag="stag")
            with (
                tc.tile_pool(name=_r + "prepps", bufs=prepb, space="PSUM") as prepps,
                tc.tile_pool(name=_r + "sigps", bufs=sigb, space="PSUM") as sigps,
                tc.tile_pool(name=_r + "blps", bufs=blb, space="PSUM") as blps,
            ):
                bls = {}
                als = {}
                emitted_batches = [0]  # batches [0, emitted) are done

                def ensure_tiles(tmax):
                    while emitted_batches[0] * TB < tmax:
                        b0 = emitted_batches[0] * TB
                        nb = min(TB, T - b0)
                        tp = prepps.tile(
                            [12, nb * 128], F16, tag="tp", name=f"{_r}tp{b0}"
                        )
                        for j in range(nb):
                            t = b0 + j
                            nc.tensor.transpose(
                                tp[:, j * 128 : (j + 1) * 128],
                                fall[:, t * 12 : (t + 1) * 12],
                                idt[:],
                            )
                        nc.vector.tensor_copy(
                            f2[:, b0 * 128 : (b0 + nb) * 128], tp[:]
                        )
                        emitted_batches[0] += 1

                dimg = d_img[:].rearrange("i c h w -> (i c) (h w)")

                def emit_blend(s):
                    al = als.pop(s)
                    off = 0
                    for t in steps[s]:
                        r, first, last = treg[t]
                        c0, cw = cwin[t]
                        nc.tensor.matmul(
                            bls[r][:]
                            .rearrange("q (h x) -> q h x", x=64)[:, :, c0 : c0 + cw],
                            c2[:, t * 12 : (t + 1) * 12],
                            al[:, off : off + 8 * cw].rearrange(
                                "p (h x) -> p h x", x=cw
                            ),
                            start=first,
                            stop=last,
                        )
                        off += 8 * cw
                        if last:
                            nc.vector.tensor_copy(
                                stag[:, r * RPX : (r + 1) * RPX], bls.pop(r)[:]
                            )
                            if r in (3, 7):  # overlap output DMA halves
                                h = 0 if r == 3 else 2048
                                nc.sync.dma_start(
                                    dimg[:, h : h + 2048],
                                    stag[:, h : h + 2048],
                                )

                g2v = g2.rearrange("k (h x) -> k h x", x=64)
                for s, tlist in enumerate(steps):
                    # transpose lookahead ahead of the sigma MMs
                    ensure_tiles(min(steps[min(s + look, len(steps) - 1)][-1] + 1, T))
                    for t in tlist:
                        r = treg[t][0]
                        if r not in bls:
                            bls[r] = blps.tile(
                                [12, RPX], F32, tag="bl", name=f"{_r}bl{r}"
                            )
                    w = sum(8 * cwin[t][1] for t in tlist)
                    sps = sigps.tile([128, w], F32, tag="sig", name=f"{_r}sig{s}")
                    off = 0
                    for t in tlist:
                        r = treg[t][0]
                        c0, cw = cwin[t]
                        nc.tensor.matmul(
                            sps[:, off : off + 8 * cw].rearrange(
                                "p (h x) -> p h x", x=cw
                            ),
                            f2[:, t * 128 : (t + 1) * 128],
                            g2v[:, 8 * r : 8 * r + 8, c0 : c0 + cw],
                            start=True,
                            stop=True,
                        )
                        off += 8 * cw
                    al = alphap.tile([128, w], F16, tag="al", name=f"{_r}al{s}")
                    nc.scalar.activation(al[:], sps[:], AF.Exp, scale=0.5)
                    als[s] = al
                    if s > 0:
                        emit_blend(s - 1)
                emit_blend(len(steps) - 1)

    nc.compile()
    return nc


_NC_CACHE = {}


def _get_program(tiles_r, **kw):
    key = (tuple(tiles_r), tuple(sorted(kw.items())))
    if key not in _NC_CACHE:
        _NC_CACHE[key] = build_program(tiles_r, **kw)
    return _NC_CACHE[key]


def make_in_maps(data, opacity, tiles_r):
    data = np.ascontiguousarray(np.asarray(data, dtype=np.float32))
    opacity = np.ascontiguousarray(np.asarray(opacity, dtype=np.float32))
    G2, ident = host_constants()
    T = sum(tiles_r)
    fp = footprints(data)

    in_maps = []
    for c in range(N_CORES):
        d8 = np.zeros((128, T * 8), np.float32)
        msk = np.zeros((128, T * 12), np.float16)
        base = 0
        for r in range(NREG):
            nt = tiles_r[r]
            slots = region_slots(data, c, r, fp)
            assert len(slots) <= nt * 128, (c, r, len(slots), nt)
            for s_idx, (i, g) in enumerate(slots):
                t = base + s_idx // 128
                p = s_idx % 128
                d8[p, t * 8 : (t + 1) * 8] = data[c * B_CORE + i, g]
                msk[p, t * 12 + 3 * i : t * 12 + 3 * i + 3] = opacity[g, 0]
            base += nt
        in_maps.append(
            {"data": d8, "mask": msk, "gconst": G2, "ident": ident}
        )
    return in_maps


def kernel(data, opacity):
    data = np.asarray(data, dtype=np.float32)
    opacity = np.asarray(opacity, dtype=np.float32)
    tiles_r, cwin = layout(data)
    nc = _get_program(tiles_r, cwin=cwin)
    in_maps = make_in_maps(data, opacity, tiles_r)
    res = bass_utils.run_bass_kernel_spmd(nc, in_maps, core_ids=list(range(N_CORES)))
    out = np.concatenate(
        [res.results[c]["img"] for c in range(N_CORES)], axis=0
    ).astype(np.float32)
    return out
